# revision 61
# baseline (speedup 1.0000x reference)
"""Causal multi-head attention (B=4, S=2048, E=768, N=12 heads, H=64) on 8
Trainium2 NeuronCores.

Sharding: core c handles batch c//2 and heads (c%2)*6 .. +6 (tensor parallel
over heads within a batch pair). No collectives: each core emits a partial
out^T = (sum over its 6 heads of z @ W_O) + b_O/2, and the host sums the two
partials per batch and transposes back.

Layout: all device math runs in a transposed layout (seq on the free axis):
  xT [E, S] per batch (host-transposed)
  Q^T/K^T per head-pair  [128 (2x64h), S] in per-512-column tiles
  V natural [S, 65*6]  (65th column per head is all-ones -> PV matmul row 64
                        accumulates the softmax denominator for free; the
                        ones columns are memset once, V projection writes
                        only the 64 value columns through a strided AP)
  S^T [k, q] scores, both heads of a pair computed concurrently in the PE
  array via tile_position row groups; on diagonal blocks the moving range of
  QK/PV and the exp width are restricted to the causally-live columns and
  only the 128-wide triangle chunk is masked (one [128,2,128] DVE multiply
  against a host tri-mask); P = exp(scale*S^T); z^T [64, q] normalized by
  1/denominator (fast DVE reciprocal + gpsimd partition_broadcast);
  out^T [E, S] accumulated over head pairs (K=128 contraction).

Engine budget: ACT runs the exp stream plus the K/Q copy-outs (Iden with a
fused bias read from the bias bundle); V and out-projection copy-outs run on
DVE; b_V is folded into b_O on the host
(out += P@(v+bV)/den @ Wo == out + Wo@bV since sum(P)/den==1).
Input DMAs issue only from the sync (xT) and gpsimd (weights) queues so the
scalar queue never head-of-line blocks the ACT stream.

Scheduling: projection blocks for query block qb+1 and the output projection
for qb are emitted as single-instruction closures drained into attention(qb+1)
iterations, filling PE bubbles left by the ACT-bound exp pipeline.
"""

import sys

sys.path.insert(0, "/opt/trn_rl_repo")

import numpy as np

B, S, E = 4, 2048, 768
N_HEADS, H = 12, 64
HPC = 6           # heads per core
PAIRS = 3         # head pairs per core
EC = E // 128     # 6 e-chunks
QB = 512          # query block (free dim of most matmuls)
NQB = S // QB     # 4
KB = 128          # key sub-block (partition dim of S^T)
SC = S // 128     # 16 s-chunks for V
VW = 65           # V width per head incl. ones column
VH = 64           # value columns per head
SCALE = 1.0 / np.sqrt(np.float32(H))

COMPUTE_DT = "bfloat16"
DEBUG = False          # adds intermediate-dump outputs

_g = {"nc": None}


def _np_dt():
    if COMPUTE_DT == "bfloat16":
        import ml_dtypes

        return ml_dtypes.bfloat16
    return np.float32


def _build(num_devices=8):
    from concourse import bacc, tile, mybir

    F32 = mybir.dt.float32
    DT = getattr(mybir.dt, COMPUTE_DT)

    nc = bacc.Bacc("TRN2", target_bir_lowering=False, debug=False,
                   num_devices=num_devices)

    d_xT = nc.dram_tensor("xT", [E, S], DT, kind="ExternalInput").ap()
    d_wq = nc.dram_tensor("wq", [PAIRS * 128, E], DT, kind="ExternalInput").ap()
    d_wk = nc.dram_tensor("wk", [PAIRS * 128, E], DT, kind="ExternalInput").ap()
    d_wv = nc.dram_tensor("wv", [128, VH * HPC * EC], DT, kind="ExternalInput").ap()
    d_wo = nc.dram_tensor("wo", [PAIRS * 128, E], DT, kind="ExternalInput").ap()
    # bundle cols: 0-2 bQ per pair, 3-5 bK per pair, 6-11 effective bO per e
    d_bundle = nc.dram_tensor("bundle", [128, 12], F32, kind="ExternalInput").ap()
    d_mask = nc.dram_tensor("mask", [KB, 2 * KB], DT, kind="ExternalInput").ap()
    d_out = nc.dram_tensor("outT", [E, S], F32, kind="ExternalOutput").ap()
    d_dbg = {}
    if DEBUG:
        for nm, shp, dtp in [("kt00", [128, QB], DT), ("qt00", [128, QB], DT),
                             ("vt0", [128, VW * HPC], DT),
                             ("pt_q0p0k0", [KB, 2 * QB], DT),
                             ("pt_q0p0k1", [KB, 2 * QB], DT),
                             ("zsb_q0", [64, 6 * QB], DT),
                             ("den_q0", [1, 6 * QB], F32),
                             ("zp_q0", [128, 3 * QB], DT)]:
            d_dbg[nm] = nc.dram_tensor(nm, shp, dtp,
                                       kind="ExternalOutput").ap()

    Exp = mybir.ActivationFunctionType.Exp
    Iden = mybir.ActivationFunctionType.Identity

    with tile.TileContext(nc) as tc:
        with tc.tile_pool(name="persist", bufs=1) as pp, \
             tc.tile_pool(name="work", bufs=4) as wp, \
             tc.tile_pool(name="zsb", bufs=3) as zp, \
             tc.tile_pool(name="outsb", bufs=4) as op, \
             tc.tile_pool(name="psA", bufs=1, space="PSUM") as psA:

            # ---- input DMAs --------------------------------------------------
            # The two first-needed pieces land in parallel: x quarter-0 heads
            # the sync HWDGE ring while wk0/bundle/wq0 head the gpsimd SWDGE
            # queue. The scalar queue issues no DMAs so ACT is never
            # head-of-line blocked behind ring flow control.
            wk0 = pp.tile([128, E], DT, tag="wk0", name="wk0")
            nc.gpsimd.dma_start(wk0[:], d_wk[0:128, :])
            bundle = pp.tile([128, 12], F32, tag="bundle", name="bundle")
            nc.gpsimd.dma_start(bundle[:], d_bundle[:, :])
            wq0 = pp.tile([128, E], DT, tag="wq0", name="wq0")
            nc.gpsimd.dma_start(wq0[:], d_wq[0:128, :])
            wk12 = pp.tile([128, 2 * E], DT, tag="wk12", name="wk12")
            nc.gpsimd.dma_start(
                wk12[:].rearrange("p (c e) -> p c e", c=2),
                d_wk[128:384, :].rearrange("(c p) e -> p c e", p=128))
            wq12 = pp.tile([128, 2 * E], DT, tag="wq12", name="wq12")
            nc.gpsimd.dma_start(
                wq12[:].rearrange("p (c e) -> p c e", c=2),
                d_wq[128:384, :].rearrange("(c p) e -> p c e", p=128))
            masksb = pp.tile([KB, 2 * KB], DT, tag="mask", name="masksb")
            nc.gpsimd.dma_start(masksb[:], d_mask[:, :])
            wv_all = pp.tile([128, VH * HPC * EC], DT, tag="wv", name="wv_all")
            nc.gpsimd.dma_start(wv_all[:], d_wv[:, :])
            wo_all = pp.tile([128, PAIRS * E], DT, tag="wo", name="wo_all")
            nc.gpsimd.dma_start(
                wo_all[:].rearrange("p (c e) -> p c e", c=PAIRS),
                d_wo[:, :].rearrange("(c p) e -> p c e", p=128))

            wk = [wk0, wk12[:, 0:E], wk12[:, E:2 * E]]
            wq = [wq0, wq12[:, 0:E], wq12[:, E:2 * E]]
            wo = [wo_all[:, p * E:(p + 1) * E] for p in range(PAIRS)]
            # partition-0 copy of wo[2] rows 64-127 so pass2's second half can
            # run in PE row group 0 (serialized with the first half — avoids
            # a concurrent-accumulate drain race into the same PSUM)
            wo2hi = pp.tile([128, E], DT, tag="wo2hi", name="wo2hi")
            nc.vector.tensor_copy(wo2hi[0:64, :], wo_all[64:128, 2 * E:3 * E])
            wv = [wv_all[:, e * VH * HPC:(e + 1) * VH * HPC] for e in range(EC)]
            mask3 = masksb[:].rearrange("p (h s) -> p h s", h=2)

            # xT: one DMA per (quarter, half-of-e-chunks); chains depend on
            # 3-chunk halves so the first matmul waits on ~1.2MB, not 3MB.
            xq = [[None, None] for _ in range(4)]
            for quarter in range(4):
                for half in range(2):
                    t = pp.tile([128, 3 * QB], DT, tag=f"xq{quarter}_{half}",
                                name=f"xq{quarter}_{half}")
                    src = d_xT[half * 3 * 128:(half * 3 + 3) * 128,
                               quarter * QB:(quarter + 1) * QB]
                    nc.sync.dma_start(
                        t[:].rearrange("p (c s) -> p c s", c=3),
                        src.rearrange("(c p) s -> p c s", p=128))
                    xq[quarter][half] = t

            # HAM warm-up: ~3.5us of dummy matmuls during the input-DMA wait
            # so the real stream starts at 2.4GHz instead of the cold 1.2.
            warm = pp.tile([128, QB], DT, tag="warm", name="warm")
            nc.vector.memset(warm[:], 0.0)
            for i in range(8):
                wps = psA.tile([128, QB], F32, tag="misc", bufs=2,
                               name=f"warm{i}")
                nc.tensor.matmul(wps[:], warm[:, 0:128], warm[:],
                                 start=True, stop=True)

            def xchunk(e, sb, lo=0, w=QB):
                # [128, w] slice of e-chunk e, query block sb
                base = (e % 3) * QB + lo
                return xq[sb][e // 3][:, base:base + w]

            kt = [[pp.tile([128, QB], DT, tag=f"kt{p}_{sb}", name=f"kt{p}_{sb}")
                   for sb in range(NQB)] for p in range(PAIRS)]
            qt = [[pp.tile([128, QB], DT, tag=f"qt{p}_{sb}", name=f"qt{p}_{sb}")
                   for sb in range(NQB)] for p in range(PAIRS)]
            vt = [pp.tile([128, VW * HPC], DT, tag=f"vt{s}", name=f"vt{s}")
                  for s in range(SC)]
            # ones columns for the denominator trick: memset whole V tiles to
            # 1.0 once; projections only ever write the 64 value columns.
            for s in range(SC):
                nc.vector.memset(vt[s][:], 1.0)

            def _mk_chain():
                def chain(name, width, lhs_of_e, rhs_of_e, copy_out):
                    st = {}
                    def mk(e):
                        def step():
                            if e == 0:
                                st["ps"] = psA.tile(
                                    [128, width], F32, tag="misc", bufs=2,
                                    name=name)
                            nc.tensor.matmul(st["ps"][:],
                                             lhs_of_e(e), rhs_of_e(e),
                                             start=(e == 0), stop=(e == EC - 1))
                        return step
                    for e in range(EC):
                        yield mk(e)
                    yield lambda: copy_out(st["ps"])
                return chain

            def kq_pair_ops(sb, p, chain=None):
                chain = chain or _mk_chain()
                # split each copy-out: left half on ACT, right half on DVE
                # in parallel — trims the saturated ACT stream by ~150ns/copy
                def kcopy(ps, p=p, sb=sb):
                    nc.scalar.activation(kt[p][sb][:, 0:256], ps[:, 0:256],
                                         Iden, bias=bundle[:, 3 + p:4 + p])
                    nc.vector.tensor_scalar_add(
                        kt[p][sb][:, 256:QB], ps[:, 256:QB],
                        bundle[:, 3 + p:4 + p])
                def qcopy(ps, p=p, sb=sb):
                    nc.scalar.activation(qt[p][sb][:, 0:256], ps[:, 0:256],
                                         Iden, bias=bundle[:, p:p + 1])
                    nc.vector.tensor_scalar_add(
                        qt[p][sb][:, 256:QB], ps[:, 256:QB],
                        bundle[:, p:p + 1])
                yield from chain(
                    f"kps{p}_{sb}", QB,
                    lambda e, p=p: wk[p][:, e * 128:(e + 1) * 128],
                    lambda e, sb=sb: xchunk(e, sb), kcopy)
                yield from chain(
                    f"qps{p}_{sb}", QB,
                    lambda e, p=p: wq[p][:, e * 128:(e + 1) * 128],
                    lambda e, sb=sb: xchunk(e, sb), qcopy)

            def kq_ops(sb, chain=None):
                for p in range(PAIRS):
                    yield from kq_pair_ops(sb, p, chain)

            def v_ops(sb, chain=None):
                chain = chain or _mk_chain()
                for s in range(4 * sb, 4 * sb + 4):
                    def vcopy(ps, s=s):
                        dst = vt[s][:].rearrange(
                            "p (h w) -> p h w", w=VW)[:, :, 0:VH]
                        nc.vector.tensor_copy(
                            dst, ps[:].rearrange("p (h w) -> p h w", w=VH))
                    yield from chain(
                        f"vps{s}", VH * HPC,
                        lambda e, sb=sb, s=s: xchunk(e, sb, (s % 4) * 128, 128),
                        lambda e: wv[e], vcopy)

            def proj_ops(sb):
                yield from kq_ops(sb)
                yield from v_ops(sb)

            def proj_block(sb):
                for step in proj_ops(sb):
                    step()

            def make_normalize(qb, zpair):
                F32R = mybir.dt.float32r
                def normalize(head, zsb, den, last=False):
                    # den is a partition-0 tile: reciprocal_approx_fast is a
                    # custom DVE op that misreads partition-offset inputs on HW
                    p, sub = head // 2, head % 2
                    hsl = slice(sub * 64, sub * 64 + 64)
                    recipf = wp.tile([1, QB], F32, tag="recipf",
                                     name=f"recipf{qb}_{head}")
                    nc.vector.reciprocal_approx_fast(recipf[:], den[:])
                    bcast = wp.tile([64, QB], F32, tag="bcast",
                                    name=f"bcast{qb}_{head}")
                    nc.gpsimd.partition_broadcast(bcast[:], recipf[:])
                    zt = zpair[p]
                    # last pair: per-head tiles (rows 0-63 each) so pass2's
                    # first contraction half starts before head 5 normalizes
                    dst = zt[sub][0:64, :] if isinstance(zt, list) else zt[hsl, :]
                    nc.vector.tensor_mul(dst, zsb[0:64, :], bcast[:])
                return normalize

            def attention(qb, drain=None, late=None, last_pair_drain=None,
                          zpair_override=None):
                nkb = 4 * qb + 4
                dq = list(drain) if drain is not None else []
                iters = [PAIRS * max(nkb - 1, 1), 0]

                def drain_some():
                    if not dq:
                        return
                    n = max(1, -(-len(dq) // max(iters[0] - iters[1], 1)))
                    for _ in range(n):
                        if dq:
                            dq.pop(0)()
                    iters[1] += 1
                zpair = zpair_override or [
                    zp.tile([128, QB], DT, tag=f"zp{p}", name=f"zp{p}_{qb}")
                    for p in range(PAIRS)]
                normalize = make_normalize(qb, zpair)
                pending = []
                for p in range(PAIRS):
                    zab = [psA.tile([VW, QB], F32, tag="z", bufs=2,
                                    name=f"zps{qb}_{2 * p + s}") for s in range(2)]

                    def qk(kb):
                        # both heads of the pair, concurrent via PE row groups;
                        # on diagonal blocks only the causally-live columns.
                        o = kb - 4 * qb
                        lo = o * 128 if o > 0 else 0
                        sps = psA.tile([KB, 2 * QB], F32, tag="s", bufs=2,
                                       name=f"sps{qb}_{p}_{kb}")
                        ktt = kt[p][kb // 4]
                        ksl = slice((kb % 4) * KB, (kb % 4 + 1) * KB)
                        nc.tensor.matmul(
                            sps[:, lo:QB], ktt[0:64, ksl],
                            qt[p][qb][0:64, lo:QB],
                            start=True, stop=True, tile_position=(0, 0))
                        nc.tensor.matmul(
                            sps[:, QB + lo:2 * QB], ktt[64:128, ksl],
                            qt[p][qb][64:128, lo:QB],
                            start=True, stop=True, tile_position=(64, 0))
                        return sps, lo

                    def pv(kb, sps, lo):
                        o = kb - 4 * qb
                        pt = wp.tile([KB, 2 * QB], DT, tag="p",
                                     name=f"pt{qb}_{p}_{kb}")
                        if lo == 0:
                            nc.scalar.activation(pt[:], sps[:], Exp,
                                                 scale=float(SCALE))
                        else:
                            s3 = sps[:].rearrange(
                                "p (h s) -> p h s", h=2)[:, :, lo:]
                            p3 = pt[:].rearrange(
                                "p (h s) -> p h s", h=2)[:, :, lo:]
                            nc.scalar.activation(p3, s3, Exp,
                                                 scale=float(SCALE))
                        if o >= 0:  # diagonal: mask the 128-wide tri chunk
                            pm = pt[:].rearrange(
                                "p (h s) -> p h s", h=2)[:, :, o * 128:(o + 1) * 128]
                            nc.vector.tensor_mul(pm, pm, mask3)
                        if DEBUG and qb == 0 and p == 0 and kb in (0, 1):
                            nc.sync.dma_start(d_dbg[f"pt_q0p0k{kb}"][:, :],
                                              pt[:])
                        for s in range(2):
                            nc.tensor.matmul(
                                zab[s][:, lo:QB],
                                vt[kb][:, (2 * p + s) * VW:(2 * p + s + 1) * VW],
                                pt[:, s * QB + lo:(s + 1) * QB],
                                start=(kb == 0), stop=(kb == nkb - 1))

                    prev = qk(0)
                    for kb in range(1, nkb):
                        cur = qk(kb)
                        pv(kb - 1, *prev)
                        drain_some()
                        prev = cur
                        if kb == 2:
                            for args in pending:
                                normalize(*args)
                            pending = []
                            if p == PAIRS - 1 and last_pair_drain is not None:
                                dq.extend(last_pair_drain)
                    pv(nkb - 1, *prev)
                    drain_some()

                    last = (qb == NQB - 1 and p == PAIRS - 1)
                    zsbs, dens = [], []
                    for s in range(2):
                        head = 2 * p + s
                        zsbs.append(wp.tile([VW, QB], DT, tag="zc", bufs=6,
                                            name=f"zsb{qb}_{head}"))
                        dens.append(wp.tile([1, QB], F32, tag="den", bufs=6,
                                            name=f"den{qb}_{head}"))
                    # one [65,512] bf16 copy per head releases the PSUM zab
                    # ~1.5us sooner (next pair PV is gated on it); the f32 den
                    # row is extracted from SBUF off that critical path
                    if last:
                        # tail: ACT is free once the exp stream ends
                        for s in range(2):
                            nc.scalar.activation(zsbs[s][:], zab[s][:], Iden)
                    else:
                        for s in range(2):
                            nc.vector.tensor_copy(zsbs[s][:], zab[s][:])
                    for s in range(2):
                        nc.vector.tensor_copy(dens[s][:], zsbs[s][64:65, :])
                    for s in range(2):
                        head = 2 * p + s
                        if DEBUG and qb == 0:
                            nc.sync.dma_start(
                                d_dbg["zsb_q0"][:, head * QB:(head + 1) * QB],
                                zsbs[s][:])
                            nc.sync.dma_start(
                                d_dbg["den_q0"][:, head * QB:(head + 1) * QB],
                                dens[s][:])
                        pending.append((head, zsbs[s], dens[s], last))
                for args in pending:
                    normalize(*args)
                if DEBUG and qb == 0:
                    nc.sync.dma_start(d_dbg["kt00"][:, :], kt[0][0][:])
                    nc.sync.dma_start(d_dbg["qt00"][:, :], qt[0][0][:])
                    nc.sync.dma_start(d_dbg["vt0"][:, :], vt[0][:])
                    for p_ in range(PAIRS):
                        nc.sync.dma_start(
                            d_dbg["zp_q0"][:, p_ * QB:(p_ + 1) * QB],
                            zpair[p_][:])
                while dq:
                    dq.pop(0)()
                if late is not None:
                    for step in late:
                        step()
                return outproj_ops(qb, zpair)

            def outproj_split(qb, zpair):
                """qb=3 variant: p0+p1 partials run early (PE bubbles during
                the last pair), only the short p2 pass waits on the final
                normalize."""
                qsl = slice(qb * QB, (qb + 1) * QB)
                partial = [None] * EC

                def pass1():
                    for e in range(EC):
                        st = {}
                        def mk(e, p, st=st):
                            def step():
                                if p == 0:
                                    st["ps"] = psA.tile(
                                        [128, QB], F32, tag="misc", bufs=2,
                                        name=f"opsa{qb}_{e}")
                                nc.tensor.matmul(
                                    st["ps"][:], wo[p][:, e * 128:(e + 1) * 128],
                                    zpair[p][:], start=(p == 0), stop=(p == 1))
                            return step
                        yield mk(e, 0)
                        yield mk(e, 1)
                        def fin(e, st=st):
                            def step():
                                t = op.tile([128, QB], F32, tag=f"partial{e}",
                                            bufs=1, name=f"partial{qb}_{e}")
                                partial[e] = t
                                nc.vector.tensor_copy(t[:], st["ps"][:])
                            return step
                        yield fin(e)

                def pass2():
                    # split by head: the head-4 half of the contraction runs
                    # while head 5 is still normalizing
                    sts = {}
                    def mka(e):
                        def step():
                            sts[e] = psA.tile([128, QB], F32, tag="misc",
                                              bufs=2, name=f"opsb{qb}_{e}")
                            nc.tensor.matmul(
                                sts[e][:], wo[2][0:64, e * 128:(e + 1) * 128],
                                zpair[2][0][0:64, :], start=True, stop=False,
                                tile_position=(0, 0))
                        return step
                    def mkb(e):
                        def step():
                            nc.tensor.matmul(
                                sts[e][:], wo2hi[0:64, e * 128:(e + 1) * 128],
                                zpair[2][1][0:64, :], start=False, stop=True,
                                tile_position=(0, 0))
                        return step
                    def fin(e):
                        def step():
                            osb = op.tile([128, QB], F32, tag="osb",
                                          name=f"osb{qb}_{e}")
                            nc.vector.scalar_tensor_tensor(
                                osb[:], sts[e][:], bundle[:, 6 + e:7 + e],
                                partial[e][:],
                                op0=mybir.AluOpType.add,
                                op1=mybir.AluOpType.add)
                            nc.sync.dma_start(
                                d_out[e * 128:(e + 1) * 128, qsl], osb[:])
                        return step
                    yield mka(0)
                    yield mka(1)
                    for e in range(EC):
                        yield mkb(e)
                        yield fin(e)
                        if e + 2 < EC:
                            yield mka(e + 2)
                return pass1, pass2

            def outproj_ops(qb, zpair):
                qsl = slice(qb * QB, (qb + 1) * QB)
                for e in range(EC):
                    st = {}
                    def mk(e, p):
                        def step():
                            if p == 0:
                                st["ps"] = psA.tile([128, QB], F32, tag="misc",
                                                    bufs=2, name=f"ops{qb}_{e}")
                            nc.tensor.matmul(
                                st["ps"][:], wo[p][:, e * 128:(e + 1) * 128],
                                zpair[p][:],
                                start=(p == 0), stop=(p == PAIRS - 1))
                        return step
                    for p in range(PAIRS):
                        yield mk(e, p)
                    def fin(e):
                        def step():
                            osb = op.tile([128, QB], F32, tag="osb",
                                          name=f"osb{qb}_{e}")
                            nc.vector.tensor_scalar_add(
                                osb[:], st["ps"][:], bundle[:, 6 + e:7 + e])
                            nc.sync.dma_start(d_out[e * 128:(e + 1) * 128, qsl],
                                              osb[:])
                        return step
                    yield fin(e)

            proj_block(0)
            carry = []
            for qb in range(NQB):
                if qb + 1 < NQB:
                    drain = list(carry) + list(kq_ops(qb + 1))
                    oops = attention(qb, drain=iter(drain), late=v_ops(qb + 1))
                    carry = list(oops)
                else:
                    zpair_last = [zp.tile([128, QB], DT, tag=f"zp{p}",
                                          name=f"zpL{p}") for p in range(2)]
                    zpair_last.append(
                        [zp.tile([128, QB], DT, tag="zp2h", bufs=2,
                                 name=f"zpL2_{s}") for s in range(2)])
                    pass1, pass2 = outproj_split(qb, zpair_last)
                    attention(qb, drain=iter(carry),
                              last_pair_drain=pass1(),
                              zpair_override=zpair_last)
                    for step in pass2():
                        step()

    nc.compile()
    return nc


def _get_nc():
    if _g["nc"] is None:
        _g["nc"] = _build()
    return _g["nc"]


def _make_in_maps(inputs):
    x = np.asarray(inputs["normalized_resid_pre"], dtype=np.float32)
    W_Q = np.asarray(inputs["W_Q"], dtype=np.float32)
    W_K = np.asarray(inputs["W_K"], dtype=np.float32)
    W_V = np.asarray(inputs["W_V"], dtype=np.float32)
    W_O = np.asarray(inputs["W_O"], dtype=np.float32)
    b_Q = np.asarray(inputs["b_Q"], dtype=np.float32)
    b_K = np.asarray(inputs["b_K"], dtype=np.float32)
    b_V = np.asarray(inputs["b_V"], dtype=np.float32)
    b_O = np.asarray(inputs["b_O"], dtype=np.float32)
    dt = _np_dt()

    # 0/1 keep-mask for the 128-wide diagonal triangle chunk, duplicated for
    # the two heads of a pair: keep when k-within-chunk <= q-within-chunk.
    tri = np.tril(np.ones((KB, KB), dtype=np.float32)).T  # [dk, dq] keep dk<=dq
    mask = np.concatenate([tri, tri], axis=1).astype(dt)  # [128, 256]

    in_maps = []
    for c in range(8):
        b = c // 2
        hs = (c % 2) * HPC
        heads = list(range(hs, hs + HPC))
        def pack(w):
            # [E, C] -> [128, EC*C] with column block e holding rows e*128..
            C = w.shape[1]
            return np.ascontiguousarray(
                w.reshape(EC, 128, C).transpose(1, 0, 2).reshape(128, EC * C))

        wq = np.concatenate(
            [pack(np.concatenate([W_Q[heads[2 * p]], W_Q[heads[2 * p + 1]]], axis=1))
             for p in range(PAIRS)], axis=0)             # [3*128, 768]
        wk = np.concatenate(
            [pack(np.concatenate([W_K[heads[2 * p]], W_K[heads[2 * p + 1]]], axis=1))
             for p in range(PAIRS)], axis=0)
        wv = pack(np.concatenate([W_V[h] for h in heads], axis=1))  # [128, 6*384]
        wo = np.concatenate(
            [np.concatenate([W_O[heads[2 * p]], W_O[heads[2 * p + 1]]], axis=0)
             for p in range(PAIRS)], axis=0)             # [3*128, 768]

        # bias bundle [128, 12]: cols 0-2 bQ pairs, 3-5 bK pairs, 6-11 bO_eff
        bundle = np.zeros((128, 12), dtype=np.float32)
        for p in range(PAIRS):
            bundle[:, p] = np.concatenate(
                [b_Q[heads[2 * p]], b_Q[heads[2 * p + 1]]])
            bundle[:, 3 + p] = np.concatenate(
                [b_K[heads[2 * p]], b_K[heads[2 * p + 1]]])
        # fold b_V into b_O: out += sum_h Wo[h] @ bV[h]  (sum(P)/den == 1)
        bo_eff = b_O / 2.0 + np.einsum(
            "nhe,nh->e", W_O[heads], b_V[heads]).astype(np.float32)
        bundle[:, 6:12] = bo_eff.reshape(EC, 128).T

        in_maps.append({
            "xT": np.ascontiguousarray(x[b].T).astype(dt),
            "wq": wq.astype(dt), "wk": wk.astype(dt),
            "wv": wv.astype(dt), "wo": wo.astype(dt),
            "bundle": bundle,
            "mask": mask,
        })
    return in_maps


def _gather(results):
    out = np.empty((B, S, E), dtype=np.float32)
    for b in range(B):
        acc = results[2 * b]["outT"].astype(np.float32) + \
              results[2 * b + 1]["outT"].astype(np.float32)
        out[b] = acc.T
    return out


def run(inputs, trace=False):
    """Returns (output, BassKernelResults)."""
    from concourse.bass_utils import run_bass_kernel_spmd

    if trace:
        _install_ntff_shim()
    nc = _get_nc()
    in_maps = _make_in_maps(inputs)
    res = run_bass_kernel_spmd(nc, in_maps, core_ids=list(range(8)), trace=trace)
    return _gather(res.results), res


def kernel(**inputs):
    out, _ = run(inputs, trace=False)
    return out


def _install_ntff_shim():
    """The agent image's antenv lacks axon_hooks; recreate it so
    run_bass_kernel_spmd(trace=True) can capture NTFF profiles."""
    import types, ctypes, contextlib

    if "antenv.axon_hooks" in sys.modules:
        return
    so_path = "/opt/axon/libaxon_pjrt.so"
    try:
        lib = ctypes.CDLL(so_path)
        lib.axon_start_nrt_profile.argtypes = [ctypes.POINTER(ctypes.c_int64),
                                              ctypes.c_size_t]
        lib.axon_start_nrt_profile.restype = ctypes.c_int64
        lib.axon_stop_nrt_profile.argtypes = [ctypes.c_char_p]
        lib.axon_stop_nrt_profile.restype = ctypes.c_int64
    except (OSError, AttributeError):
        return

    @contextlib.contextmanager
    def _hook(output_dir, device_ids):
        import jax

        jax.devices()
        if device_ids:
            ids = (ctypes.c_int64 * len(device_ids))(*device_ids)
            rc = lib.axon_start_nrt_profile(ids, len(device_ids))
        else:
            rc = lib.axon_start_nrt_profile(None, 0)
        if rc != 0:
            raise RuntimeError(f"axon_start_nrt_profile rc={rc}")
        try:
            yield
        finally:
            n = lib.axon_stop_nrt_profile(str(output_dir).encode())
            print(f"ntff profile: {n} file(s) -> {output_dir}", file=sys.stderr)

    mod = types.ModuleType("antenv.axon_hooks")
    mod.get_axon_ntff_profile_hook = lambda: _hook
    sys.modules["antenv.axon_hooks"] = mod
    # avoid S3 upload attempts from the trace post-processing
    from concourse import bass_utils as bu

    bu.upload_artifacts = lambda tmpdir: f"local:{tmpdir}"


# revision 62
# speedup vs baseline: 1.2217x; 1.2217x over previous
"""Causal multi-head attention (B=4, S=2048, E=768, N=12 heads, H=64) on 8
Trainium2 NeuronCores.

Sharding: core c handles batch c//2 and heads (c%2)*6 .. +6 (tensor parallel
over heads within a batch pair). No collectives: each core emits a partial
out^T = (sum over its 6 heads of z @ W_O) + b_O/2, and the host sums the two
partials per batch and transposes back.

Layout: all device math runs in a transposed layout (seq on the free axis):
  xT [E, S] per batch (host-transposed)
  Q^T/K^T per head-pair  [128 (2x64h), S] in per-512-column tiles
  V natural [S, 65*6]  (65th column per head is all-ones -> PV matmul row 64
                        accumulates the softmax denominator for free; the
                        ones columns are memset once, V projection writes
                        only the 64 value columns through a strided AP)
  S^T [k, q] scores, both heads of a pair computed concurrently in the PE
  array via tile_position row groups; on diagonal blocks the moving range of
  QK/PV and the exp width are restricted to the causally-live columns and
  only the 128-wide triangle chunk is masked (one [128,2,128] DVE multiply
  against a host tri-mask); P = exp(scale*S^T); z^T [64, q] normalized by
  1/denominator (fast DVE reciprocal + gpsimd partition_broadcast);
  out^T [E, S] accumulated over head pairs (K=128 contraction).

Engine budget: ACT runs the exp stream plus the K/Q copy-outs (Iden with a
fused bias read from the bias bundle); V and out-projection copy-outs run on
DVE; b_V is folded into b_O on the host
(out += P@(v+bV)/den @ Wo == out + Wo@bV since sum(P)/den==1).
Input DMAs issue only from the sync (xT) and gpsimd (weights) queues so the
scalar queue never head-of-line blocks the ACT stream.

Scheduling: projection blocks for query block qb+1 and the output projection
for qb are emitted as single-instruction closures drained into attention(qb+1)
iterations, filling PE bubbles left by the ACT-bound exp pipeline.
"""

import sys

sys.path.insert(0, "/opt/trn_rl_repo")

import numpy as np

B, S, E = 4, 2048, 768
N_HEADS, H = 12, 64
HPC = 6           # heads per core
PAIRS = 3         # head pairs per core
EC = E // 128     # 6 e-chunks
QB = 512          # query block (free dim of most matmuls)
NQB = S // QB     # 4
KB = 128          # key sub-block (partition dim of S^T)
SC = S // 128     # 16 s-chunks for V
VW = 65           # V width per head incl. ones column
VH = 64           # value columns per head
SCALE = 1.0 / np.sqrt(np.float32(H))

COMPUTE_DT = "bfloat16"
DEBUG = False          # adds intermediate-dump outputs

_g = {"nc": None}


def _np_dt():
    if COMPUTE_DT == "bfloat16":
        import ml_dtypes

        return ml_dtypes.bfloat16
    return np.float32


def _build(num_devices=8):
    from concourse import bacc, tile, mybir

    F32 = mybir.dt.float32
    DT = getattr(mybir.dt, COMPUTE_DT)

    nc = bacc.Bacc("TRN2", target_bir_lowering=False, debug=False,
                   num_devices=num_devices)

    d_xT = nc.dram_tensor("xT", [E, S], DT, kind="ExternalInput").ap()
    d_wq = nc.dram_tensor("wq", [PAIRS * 128, E], DT, kind="ExternalInput").ap()
    d_wk = nc.dram_tensor("wk", [PAIRS * 128, E], DT, kind="ExternalInput").ap()
    d_wv = nc.dram_tensor("wv", [128, VH * HPC * EC], DT, kind="ExternalInput").ap()
    d_wo = nc.dram_tensor("wo", [PAIRS * 128, E], DT, kind="ExternalInput").ap()
    # bundle cols: 0-2 bQ per pair, 3-5 bK per pair, 6-11 effective bO per e
    d_bundle = nc.dram_tensor("bundle", [128, 12], F32, kind="ExternalInput").ap()
    d_mask = nc.dram_tensor("mask", [KB, 2 * KB], DT, kind="ExternalInput").ap()
    d_out = nc.dram_tensor("outT", [E, S], F32, kind="ExternalOutput").ap()
    d_dbg = {}
    if DEBUG:
        for nm, shp, dtp in [("kt00", [128, QB], DT), ("qt00", [128, QB], DT),
                             ("vt0", [128, VW * HPC], DT),
                             ("pt_q0p0k0", [KB, 2 * QB], DT),
                             ("pt_q0p0k1", [KB, 2 * QB], DT),
                             ("zsb_q0", [64, 6 * QB], DT),
                             ("den_q0", [1, 6 * QB], F32),
                             ("zp_q0", [128, 3 * QB], DT)]:
            d_dbg[nm] = nc.dram_tensor(nm, shp, dtp,
                                       kind="ExternalOutput").ap()

    Exp = mybir.ActivationFunctionType.Exp
    Iden = mybir.ActivationFunctionType.Identity

    with tile.TileContext(nc) as tc:
        with tc.tile_pool(name="persist", bufs=1) as pp, \
             tc.tile_pool(name="work", bufs=4) as wp, \
             tc.tile_pool(name="zsb", bufs=3) as zp, \
             tc.tile_pool(name="outsb", bufs=4) as op, \
             tc.tile_pool(name="psA", bufs=1, space="PSUM") as psA:

            # ---- input DMAs --------------------------------------------------
            # The two first-needed pieces land in parallel: x quarter-0 heads
            # the sync HWDGE ring while wk0/bundle/wq0 head the gpsimd SWDGE
            # queue. The scalar queue issues no DMAs so ACT is never
            # head-of-line blocked behind ring flow control.
            wk0 = pp.tile([128, E], DT, tag="wk0", name="wk0")
            nc.gpsimd.dma_start(wk0[:], d_wk[0:128, :])
            bundle = pp.tile([128, 12], F32, tag="bundle", name="bundle")
            nc.gpsimd.dma_start(bundle[:], d_bundle[:, :])
            wq0 = pp.tile([128, E], DT, tag="wq0", name="wq0")
            nc.gpsimd.dma_start(wq0[:], d_wq[0:128, :])
            wk12 = pp.tile([128, 2 * E], DT, tag="wk12", name="wk12")
            nc.gpsimd.dma_start(
                wk12[:].rearrange("p (c e) -> p c e", c=2),
                d_wk[128:384, :].rearrange("(c p) e -> p c e", p=128))
            wq12 = pp.tile([128, 2 * E], DT, tag="wq12", name="wq12")
            nc.gpsimd.dma_start(
                wq12[:].rearrange("p (c e) -> p c e", c=2),
                d_wq[128:384, :].rearrange("(c p) e -> p c e", p=128))
            masksb = pp.tile([KB, 2 * KB], DT, tag="mask", name="masksb")
            nc.gpsimd.dma_start(masksb[:], d_mask[:, :])
            wv_all = pp.tile([128, VH * HPC * EC], DT, tag="wv", name="wv_all")
            nc.gpsimd.dma_start(wv_all[:], d_wv[:, :])
            wo_all = pp.tile([128, PAIRS * E], DT, tag="wo", name="wo_all")
            nc.gpsimd.dma_start(
                wo_all[:].rearrange("p (c e) -> p c e", c=PAIRS),
                d_wo[:, :].rearrange("(c p) e -> p c e", p=128))

            wk = [wk0, wk12[:, 0:E], wk12[:, E:2 * E]]
            wq = [wq0, wq12[:, 0:E], wq12[:, E:2 * E]]
            wo = [wo_all[:, p * E:(p + 1) * E] for p in range(PAIRS)]
            # partition-0 copy of wo[2] rows 64-127 so pass2's second half can
            # run in PE row group 0 (serialized with the first half — avoids
            # a concurrent-accumulate drain race into the same PSUM)
            wo2hi = pp.tile([128, E], DT, tag="wo2hi", name="wo2hi")
            nc.vector.tensor_copy(wo2hi[0:64, :], wo_all[64:128, 2 * E:3 * E])
            wv = [wv_all[:, e * VH * HPC:(e + 1) * VH * HPC] for e in range(EC)]
            mask3 = masksb[:].rearrange("p (h s) -> p h s", h=2)

            # xT: one DMA per (quarter, half-of-e-chunks); chains depend on
            # 3-chunk halves so the first matmul waits on ~1.2MB, not 3MB.
            xq = [[None, None] for _ in range(4)]
            for quarter in range(4):
                for half in range(2):
                    t = pp.tile([128, 3 * QB], DT, tag=f"xq{quarter}_{half}",
                                name=f"xq{quarter}_{half}")
                    src = d_xT[half * 3 * 128:(half * 3 + 3) * 128,
                               quarter * QB:(quarter + 1) * QB]
                    nc.sync.dma_start(
                        t[:].rearrange("p (c s) -> p c s", c=3),
                        src.rearrange("(c p) s -> p c s", p=128))
                    xq[quarter][half] = t

            # HAM warm-up: ~3.5us of dummy matmuls during the input-DMA wait
            # so the real stream starts at 2.4GHz instead of the cold 1.2.
            warm = pp.tile([128, QB], DT, tag="warm", name="warm")
            nc.vector.memset(warm[:], 0.0)
            for i in range(8):
                wps = psA.tile([128, QB], F32, tag="misc", bufs=2,
                               name=f"warm{i}")
                nc.tensor.matmul(wps[:], warm[:, 0:128], warm[:],
                                 start=True, stop=True)

            def xchunk(e, sb, lo=0, w=QB):
                # [128, w] slice of e-chunk e, query block sb
                base = (e % 3) * QB + lo
                return xq[sb][e // 3][:, base:base + w]

            kt = [[pp.tile([128, QB], DT, tag=f"kt{p}_{sb}", name=f"kt{p}_{sb}")
                   for sb in range(NQB)] for p in range(PAIRS)]
            qt = [[pp.tile([128, QB], DT, tag=f"qt{p}_{sb}", name=f"qt{p}_{sb}")
                   for sb in range(NQB)] for p in range(PAIRS)]
            vt = [pp.tile([128, VW * HPC], DT, tag=f"vt{s}", name=f"vt{s}")
                  for s in range(SC)]
            # ones columns for the denominator trick: memset whole V tiles to
            # 1.0 once; projections only ever write the 64 value columns.
            for s in range(SC):
                nc.vector.memset(vt[s][:], 1.0)

            def _mk_chain():
                def chain(name, width, lhs_of_e, rhs_of_e, copy_out):
                    st = {}
                    def mk(e):
                        def step():
                            if e == 0:
                                st["ps"] = psA.tile(
                                    [128, width], F32, tag="misc", bufs=2,
                                    name=name)
                            nc.tensor.matmul(st["ps"][:],
                                             lhs_of_e(e), rhs_of_e(e),
                                             start=(e == 0), stop=(e == EC - 1))
                        return step
                    for e in range(EC):
                        yield mk(e)
                    yield lambda: copy_out(st["ps"])
                return chain

            def kq_pair_ops(sb, p, chain=None):
                chain = chain or _mk_chain()
                kcopy = lambda ps, p=p, sb=sb: nc.scalar.activation(
                    kt[p][sb][:], ps[:], Iden, bias=bundle[:, 3 + p:4 + p])
                qcopy = lambda ps, p=p, sb=sb: nc.scalar.activation(
                    qt[p][sb][:], ps[:], Iden, bias=bundle[:, p:p + 1])
                yield from chain(
                    f"kps{p}_{sb}", QB,
                    lambda e, p=p: wk[p][:, e * 128:(e + 1) * 128],
                    lambda e, sb=sb: xchunk(e, sb), kcopy)
                yield from chain(
                    f"qps{p}_{sb}", QB,
                    lambda e, p=p: wq[p][:, e * 128:(e + 1) * 128],
                    lambda e, sb=sb: xchunk(e, sb), qcopy)

            def kq_ops(sb, chain=None):
                for p in range(PAIRS):
                    yield from kq_pair_ops(sb, p, chain)

            def v_ops(sb, chain=None):
                chain = chain or _mk_chain()
                for s in range(4 * sb, 4 * sb + 4):
                    def vcopy(ps, s=s):
                        dst = vt[s][:].rearrange(
                            "p (h w) -> p h w", w=VW)[:, :, 0:VH]
                        nc.vector.tensor_copy(
                            dst, ps[:].rearrange("p (h w) -> p h w", w=VH))
                    yield from chain(
                        f"vps{s}", VH * HPC,
                        lambda e, sb=sb, s=s: xchunk(e, sb, (s % 4) * 128, 128),
                        lambda e: wv[e], vcopy)

            def proj_ops(sb):
                yield from kq_ops(sb)
                yield from v_ops(sb)

            def proj_block(sb):
                for step in proj_ops(sb):
                    step()

            def make_normalize(qb, zpair):
                F32R = mybir.dt.float32r
                def normalize(head, zsb, den, last=False):
                    # den is a partition-0 tile: reciprocal_approx_fast is a
                    # custom DVE op that misreads partition-offset inputs on HW
                    p, sub = head // 2, head % 2
                    hsl = slice(sub * 64, sub * 64 + 64)
                    recipf = wp.tile([1, QB], F32, tag="recipf",
                                     name=f"recipf{qb}_{head}")
                    nc.vector.reciprocal_approx_fast(recipf[:], den[:])
                    bcast = wp.tile([64, QB], F32, tag="bcast",
                                    name=f"bcast{qb}_{head}")
                    nc.gpsimd.partition_broadcast(bcast[:], recipf[:])
                    zt = zpair[p]
                    # last pair: per-head tiles (rows 0-63 each) so pass2's
                    # first contraction half starts before head 5 normalizes
                    dst = zt[sub][0:64, :] if isinstance(zt, list) else zt[hsl, :]
                    nc.vector.tensor_mul(dst, zsb[0:64, :], bcast[:])
                return normalize

            def attention(qb, drain=None, late=None, last_pair_drain=None,
                          zpair_override=None):
                nkb = 4 * qb + 4
                dq = list(drain) if drain is not None else []
                iters = [PAIRS * max(nkb - 1, 1), 0]

                def drain_some():
                    if not dq:
                        return
                    n = max(1, -(-len(dq) // max(iters[0] - iters[1], 1)))
                    for _ in range(n):
                        if dq:
                            dq.pop(0)()
                    iters[1] += 1
                zpair = zpair_override or [
                    zp.tile([128, QB], DT, tag=f"zp{p}", name=f"zp{p}_{qb}")
                    for p in range(PAIRS)]
                normalize = make_normalize(qb, zpair)
                pending = []
                for p in range(PAIRS):
                    zab = [psA.tile([VW, QB], F32, tag="z", bufs=2,
                                    name=f"zps{qb}_{2 * p + s}") for s in range(2)]

                    def qk(kb):
                        # both heads of the pair, concurrent via PE row groups;
                        # on diagonal blocks only the causally-live columns.
                        o = kb - 4 * qb
                        lo = o * 128 if o > 0 else 0
                        sps = psA.tile([KB, 2 * QB], F32, tag="s", bufs=2,
                                       name=f"sps{qb}_{p}_{kb}")
                        ktt = kt[p][kb // 4]
                        ksl = slice((kb % 4) * KB, (kb % 4 + 1) * KB)
                        nc.tensor.matmul(
                            sps[:, lo:QB], ktt[0:64, ksl],
                            qt[p][qb][0:64, lo:QB],
                            start=True, stop=True, tile_position=(0, 0))
                        nc.tensor.matmul(
                            sps[:, QB + lo:2 * QB], ktt[64:128, ksl],
                            qt[p][qb][64:128, lo:QB],
                            start=True, stop=True, tile_position=(64, 0))
                        return sps, lo

                    def pv(kb, sps, lo):
                        o = kb - 4 * qb
                        pt = wp.tile([KB, 2 * QB], DT, tag="p",
                                     name=f"pt{qb}_{p}_{kb}")
                        if lo == 0:
                            nc.scalar.activation(pt[:], sps[:], Exp,
                                                 scale=float(SCALE))
                        else:
                            s3 = sps[:].rearrange(
                                "p (h s) -> p h s", h=2)[:, :, lo:]
                            p3 = pt[:].rearrange(
                                "p (h s) -> p h s", h=2)[:, :, lo:]
                            nc.scalar.activation(p3, s3, Exp,
                                                 scale=float(SCALE))
                        if o >= 0:  # diagonal: mask the 128-wide tri chunk
                            pm = pt[:].rearrange(
                                "p (h s) -> p h s", h=2)[:, :, o * 128:(o + 1) * 128]
                            nc.vector.tensor_mul(pm, pm, mask3)
                        if DEBUG and qb == 0 and p == 0 and kb in (0, 1):
                            nc.sync.dma_start(d_dbg[f"pt_q0p0k{kb}"][:, :],
                                              pt[:])
                        for s in range(2):
                            nc.tensor.matmul(
                                zab[s][:, lo:QB],
                                vt[kb][:, (2 * p + s) * VW:(2 * p + s + 1) * VW],
                                pt[:, s * QB + lo:(s + 1) * QB],
                                start=(kb == 0), stop=(kb == nkb - 1))

                    prev = qk(0)
                    for kb in range(1, nkb):
                        cur = qk(kb)
                        pv(kb - 1, *prev)
                        drain_some()
                        prev = cur
                        if kb == 2:
                            for args in pending:
                                normalize(*args)
                            pending = []
                            if p == PAIRS - 1 and last_pair_drain is not None:
                                dq.extend(last_pair_drain)
                    pv(nkb - 1, *prev)
                    drain_some()

                    last = (qb == NQB - 1 and p == PAIRS - 1)
                    zsbs, dens = [], []
                    for s in range(2):
                        head = 2 * p + s
                        zsbs.append(wp.tile([VW, QB], DT, tag="zc", bufs=6,
                                            name=f"zsb{qb}_{head}"))
                        dens.append(wp.tile([1, QB], F32, tag="den", bufs=6,
                                            name=f"den{qb}_{head}"))
                    # one [65,512] bf16 copy per head releases the PSUM zab
                    # ~1.5us sooner (next pair PV is gated on it); the f32 den
                    # row is extracted from SBUF off that critical path
                    if last:
                        # tail: ACT is free once the exp stream ends
                        for s in range(2):
                            nc.scalar.activation(zsbs[s][:], zab[s][:], Iden)
                    else:
                        for s in range(2):
                            nc.vector.tensor_copy(zsbs[s][:], zab[s][:])
                    for s in range(2):
                        nc.vector.tensor_copy(dens[s][:], zsbs[s][64:65, :])
                    for s in range(2):
                        head = 2 * p + s
                        if DEBUG and qb == 0:
                            nc.sync.dma_start(
                                d_dbg["zsb_q0"][:, head * QB:(head + 1) * QB],
                                zsbs[s][:])
                            nc.sync.dma_start(
                                d_dbg["den_q0"][:, head * QB:(head + 1) * QB],
                                dens[s][:])
                        pending.append((head, zsbs[s], dens[s], last))
                for args in pending:
                    normalize(*args)
                if DEBUG and qb == 0:
                    nc.sync.dma_start(d_dbg["kt00"][:, :], kt[0][0][:])
                    nc.sync.dma_start(d_dbg["qt00"][:, :], qt[0][0][:])
                    nc.sync.dma_start(d_dbg["vt0"][:, :], vt[0][:])
                    for p_ in range(PAIRS):
                        nc.sync.dma_start(
                            d_dbg["zp_q0"][:, p_ * QB:(p_ + 1) * QB],
                            zpair[p_][:])
                while dq:
                    dq.pop(0)()
                if late is not None:
                    for step in late:
                        step()
                return outproj_ops(qb, zpair)

            def outproj_split(qb, zpair):
                """qb=3 variant: p0+p1 partials run early (PE bubbles during
                the last pair), only the short p2 pass waits on the final
                normalize."""
                qsl = slice(qb * QB, (qb + 1) * QB)
                partial = [None] * EC

                def pass1():
                    for e in range(EC):
                        st = {}
                        def mk(e, p, st=st):
                            def step():
                                if p == 0:
                                    st["ps"] = psA.tile(
                                        [128, QB], F32, tag="misc", bufs=2,
                                        name=f"opsa{qb}_{e}")
                                nc.tensor.matmul(
                                    st["ps"][:], wo[p][:, e * 128:(e + 1) * 128],
                                    zpair[p][:], start=(p == 0), stop=(p == 1))
                            return step
                        yield mk(e, 0)
                        yield mk(e, 1)
                        def fin(e, st=st):
                            def step():
                                t = op.tile([128, QB], F32, tag=f"partial{e}",
                                            bufs=1, name=f"partial{qb}_{e}")
                                partial[e] = t
                                nc.vector.tensor_copy(t[:], st["ps"][:])
                            return step
                        yield fin(e)

                def pass2():
                    # split by head: the head-4 half of the contraction runs
                    # while head 5 is still normalizing
                    sts = {}
                    def mka(e):
                        def step():
                            sts[e] = psA.tile([128, QB], F32, tag="misc",
                                              bufs=2, name=f"opsb{qb}_{e}")
                            nc.tensor.matmul(
                                sts[e][:], wo[2][0:64, e * 128:(e + 1) * 128],
                                zpair[2][0][0:64, :], start=True, stop=False,
                                tile_position=(0, 0))
                        return step
                    def mkb(e):
                        def step():
                            nc.tensor.matmul(
                                sts[e][:], wo2hi[0:64, e * 128:(e + 1) * 128],
                                zpair[2][1][0:64, :], start=False, stop=True,
                                tile_position=(0, 0))
                        return step
                    def fin(e):
                        def step():
                            osb = op.tile([128, QB], F32, tag="osb",
                                          name=f"osb{qb}_{e}")
                            nc.vector.scalar_tensor_tensor(
                                osb[:], sts[e][:], bundle[:, 6 + e:7 + e],
                                partial[e][:],
                                op0=mybir.AluOpType.add,
                                op1=mybir.AluOpType.add)
                            nc.sync.dma_start(
                                d_out[e * 128:(e + 1) * 128, qsl], osb[:])
                        return step
                    yield mka(0)
                    yield mka(1)
                    for e in range(EC):
                        yield mkb(e)
                        yield fin(e)
                        if e + 2 < EC:
                            yield mka(e + 2)
                return pass1, pass2

            def outproj_ops(qb, zpair):
                qsl = slice(qb * QB, (qb + 1) * QB)
                for e in range(EC):
                    st = {}
                    def mk(e, p):
                        def step():
                            if p == 0:
                                st["ps"] = psA.tile([128, QB], F32, tag="misc",
                                                    bufs=2, name=f"ops{qb}_{e}")
                            nc.tensor.matmul(
                                st["ps"][:], wo[p][:, e * 128:(e + 1) * 128],
                                zpair[p][:],
                                start=(p == 0), stop=(p == PAIRS - 1))
                        return step
                    for p in range(PAIRS):
                        yield mk(e, p)
                    def fin(e):
                        def step():
                            osb = op.tile([128, QB], F32, tag="osb",
                                          name=f"osb{qb}_{e}")
                            nc.vector.tensor_scalar_add(
                                osb[:], st["ps"][:], bundle[:, 6 + e:7 + e])
                            nc.sync.dma_start(d_out[e * 128:(e + 1) * 128, qsl],
                                              osb[:])
                        return step
                    yield fin(e)

            proj_block(0)
            carry = []
            for qb in range(NQB):
                if qb + 1 < NQB:
                    drain = list(carry) + list(kq_ops(qb + 1))
                    oops = attention(qb, drain=iter(drain), late=v_ops(qb + 1))
                    carry = list(oops)
                else:
                    zpair_last = [zp.tile([128, QB], DT, tag=f"zp{p}",
                                          name=f"zpL{p}") for p in range(2)]
                    zpair_last.append(
                        [zp.tile([128, QB], DT, tag="zp2h", bufs=2,
                                 name=f"zpL2_{s}") for s in range(2)])
                    pass1, pass2 = outproj_split(qb, zpair_last)
                    attention(qb, drain=iter(carry),
                              last_pair_drain=pass1(),
                              zpair_override=zpair_last)
                    for step in pass2():
                        step()

    nc.compile()
    return nc


def _get_nc():
    if _g["nc"] is None:
        _g["nc"] = _build()
    return _g["nc"]


def _make_in_maps(inputs):
    x = np.asarray(inputs["normalized_resid_pre"], dtype=np.float32)
    W_Q = np.asarray(inputs["W_Q"], dtype=np.float32)
    W_K = np.asarray(inputs["W_K"], dtype=np.float32)
    W_V = np.asarray(inputs["W_V"], dtype=np.float32)
    W_O = np.asarray(inputs["W_O"], dtype=np.float32)
    b_Q = np.asarray(inputs["b_Q"], dtype=np.float32)
    b_K = np.asarray(inputs["b_K"], dtype=np.float32)
    b_V = np.asarray(inputs["b_V"], dtype=np.float32)
    b_O = np.asarray(inputs["b_O"], dtype=np.float32)
    dt = _np_dt()

    # 0/1 keep-mask for the 128-wide diagonal triangle chunk, duplicated for
    # the two heads of a pair: keep when k-within-chunk <= q-within-chunk.
    tri = np.tril(np.ones((KB, KB), dtype=np.float32)).T  # [dk, dq] keep dk<=dq
    mask = np.concatenate([tri, tri], axis=1).astype(dt)  # [128, 256]

    in_maps = []
    for c in range(8):
        b = c // 2
        hs = (c % 2) * HPC
        heads = list(range(hs, hs + HPC))
        def pack(w):
            # [E, C] -> [128, EC*C] with column block e holding rows e*128..
            C = w.shape[1]
            return np.ascontiguousarray(
                w.reshape(EC, 128, C).transpose(1, 0, 2).reshape(128, EC * C))

        wq = np.concatenate(
            [pack(np.concatenate([W_Q[heads[2 * p]], W_Q[heads[2 * p + 1]]], axis=1))
             for p in range(PAIRS)], axis=0)             # [3*128, 768]
        wk = np.concatenate(
            [pack(np.concatenate([W_K[heads[2 * p]], W_K[heads[2 * p + 1]]], axis=1))
             for p in range(PAIRS)], axis=0)
        wv = pack(np.concatenate([W_V[h] for h in heads], axis=1))  # [128, 6*384]
        wo = np.concatenate(
            [np.concatenate([W_O[heads[2 * p]], W_O[heads[2 * p + 1]]], axis=0)
             for p in range(PAIRS)], axis=0)             # [3*128, 768]

        # bias bundle [128, 12]: cols 0-2 bQ pairs, 3-5 bK pairs, 6-11 bO_eff
        bundle = np.zeros((128, 12), dtype=np.float32)
        for p in range(PAIRS):
            bundle[:, p] = np.concatenate(
                [b_Q[heads[2 * p]], b_Q[heads[2 * p + 1]]])
            bundle[:, 3 + p] = np.concatenate(
                [b_K[heads[2 * p]], b_K[heads[2 * p + 1]]])
        # fold b_V into b_O: out += sum_h Wo[h] @ bV[h]  (sum(P)/den == 1)
        bo_eff = b_O / 2.0 + np.einsum(
            "nhe,nh->e", W_O[heads], b_V[heads]).astype(np.float32)
        bundle[:, 6:12] = bo_eff.reshape(EC, 128).T

        in_maps.append({
            "xT": np.ascontiguousarray(x[b].T).astype(dt),
            "wq": wq.astype(dt), "wk": wk.astype(dt),
            "wv": wv.astype(dt), "wo": wo.astype(dt),
            "bundle": bundle,
            "mask": mask,
        })
    return in_maps


def _gather(results):
    out = np.empty((B, S, E), dtype=np.float32)
    for b in range(B):
        acc = results[2 * b]["outT"].astype(np.float32) + \
              results[2 * b + 1]["outT"].astype(np.float32)
        out[b] = acc.T
    return out


def run(inputs, trace=False):
    """Returns (output, BassKernelResults)."""
    from concourse.bass_utils import run_bass_kernel_spmd

    if trace:
        _install_ntff_shim()
    nc = _get_nc()
    in_maps = _make_in_maps(inputs)
    res = run_bass_kernel_spmd(nc, in_maps, core_ids=list(range(8)), trace=trace)
    return _gather(res.results), res


def kernel(**inputs):
    out, _ = run(inputs, trace=False)
    return out


def _install_ntff_shim():
    """The agent image's antenv lacks axon_hooks; recreate it so
    run_bass_kernel_spmd(trace=True) can capture NTFF profiles."""
    import types, ctypes, contextlib

    if "antenv.axon_hooks" in sys.modules:
        return
    so_path = "/opt/axon/libaxon_pjrt.so"
    try:
        lib = ctypes.CDLL(so_path)
        lib.axon_start_nrt_profile.argtypes = [ctypes.POINTER(ctypes.c_int64),
                                              ctypes.c_size_t]
        lib.axon_start_nrt_profile.restype = ctypes.c_int64
        lib.axon_stop_nrt_profile.argtypes = [ctypes.c_char_p]
        lib.axon_stop_nrt_profile.restype = ctypes.c_int64
    except (OSError, AttributeError):
        return

    @contextlib.contextmanager
    def _hook(output_dir, device_ids):
        import jax

        jax.devices()
        if device_ids:
            ids = (ctypes.c_int64 * len(device_ids))(*device_ids)
            rc = lib.axon_start_nrt_profile(ids, len(device_ids))
        else:
            rc = lib.axon_start_nrt_profile(None, 0)
        if rc != 0:
            raise RuntimeError(f"axon_start_nrt_profile rc={rc}")
        try:
            yield
        finally:
            n = lib.axon_stop_nrt_profile(str(output_dir).encode())
            print(f"ntff profile: {n} file(s) -> {output_dir}", file=sys.stderr)

    mod = types.ModuleType("antenv.axon_hooks")
    mod.get_axon_ntff_profile_hook = lambda: _hook
    sys.modules["antenv.axon_hooks"] = mod
    # avoid S3 upload attempts from the trace post-processing
    from concourse import bass_utils as bu

    bu.upload_artifacts = lambda tmpdir: f"local:{tmpdir}"


# revision 63
# speedup vs baseline: 1.2306x; 1.0073x over previous
"""Causal multi-head attention (B=4, S=2048, E=768, N=12 heads, H=64) on 8
Trainium2 NeuronCores.

Sharding: core c handles batch c//2 and heads (c%2)*6 .. +6 (tensor parallel
over heads within a batch pair). No collectives: each core emits a partial
out^T = (sum over its 6 heads of z @ W_O) + b_O/2, and the host sums the two
partials per batch and transposes back.

Layout: all device math runs in a transposed layout (seq on the free axis):
  xT [E, S] per batch (host-transposed)
  Q^T/K^T per head-pair  [128 (2x64h), S] in per-512-column tiles
  V natural [S, 65*6]  (65th column per head is all-ones -> PV matmul row 64
                        accumulates the softmax denominator for free; the
                        ones columns are memset once, V projection writes
                        only the 64 value columns through a strided AP)
  S^T [k, q] scores, both heads of a pair computed concurrently in the PE
  array via tile_position row groups; on diagonal blocks the moving range of
  QK/PV and the exp width are restricted to the causally-live columns and
  only the 128-wide triangle chunk is masked (one [128,2,128] DVE multiply
  against a host tri-mask); P = exp(scale*S^T); z^T [64, q] normalized by
  1/denominator (fast DVE reciprocal + gpsimd partition_broadcast);
  out^T [E, S] accumulated over head pairs (K=128 contraction).

Engine budget: ACT runs the exp stream plus the K/Q copy-outs (Iden with a
fused bias read from the bias bundle); V and out-projection copy-outs run on
DVE; b_V is folded into b_O on the host
(out += P@(v+bV)/den @ Wo == out + Wo@bV since sum(P)/den==1).
Input DMAs issue only from the sync (xT) and gpsimd (weights) queues so the
scalar queue never head-of-line blocks the ACT stream.

Scheduling: projection blocks for query block qb+1 and the output projection
for qb are emitted as single-instruction closures drained into attention(qb+1)
iterations, filling PE bubbles left by the ACT-bound exp pipeline.
"""

import sys

sys.path.insert(0, "/opt/trn_rl_repo")

import numpy as np

B, S, E = 4, 2048, 768
N_HEADS, H = 12, 64
HPC = 6           # heads per core
PAIRS = 3         # head pairs per core
EC = E // 128     # 6 e-chunks
QB = 512          # query block (free dim of most matmuls)
NQB = S // QB     # 4
KB = 128          # key sub-block (partition dim of S^T)
SC = S // 128     # 16 s-chunks for V
VW = 65           # V width per head incl. ones column
VH = 64           # value columns per head
SCALE = 1.0 / np.sqrt(np.float32(H))

COMPUTE_DT = "bfloat16"
DEBUG = False          # adds intermediate-dump outputs

_g = {"nc": None}


def _np_dt():
    if COMPUTE_DT == "bfloat16":
        import ml_dtypes

        return ml_dtypes.bfloat16
    return np.float32


def _build(num_devices=8):
    from concourse import bacc, tile, mybir

    F32 = mybir.dt.float32
    DT = getattr(mybir.dt, COMPUTE_DT)

    nc = bacc.Bacc("TRN2", target_bir_lowering=False, debug=False,
                   num_devices=num_devices)

    d_xT = nc.dram_tensor("xT", [E, S], DT, kind="ExternalInput").ap()
    d_wq = nc.dram_tensor("wq", [PAIRS * 128, E], DT, kind="ExternalInput").ap()
    d_wk = nc.dram_tensor("wk", [PAIRS * 128, E], DT, kind="ExternalInput").ap()
    d_wv = nc.dram_tensor("wv", [128, VH * HPC * EC], DT, kind="ExternalInput").ap()
    d_wo = nc.dram_tensor("wo", [PAIRS * 128, E], DT, kind="ExternalInput").ap()
    # bundle cols: 0-2 bQ per pair, 3-5 bK per pair, 6-11 effective bO per e
    d_bundle = nc.dram_tensor("bundle", [128, 12], F32, kind="ExternalInput").ap()
    d_mask = nc.dram_tensor("mask", [KB, 2 * KB], DT, kind="ExternalInput").ap()
    d_out = nc.dram_tensor("outT", [E, S], F32, kind="ExternalOutput").ap()
    d_dbg = {}
    if DEBUG:
        for nm, shp, dtp in [("kt00", [128, QB], DT), ("qt00", [128, QB], DT),
                             ("vt0", [128, VW * HPC], DT),
                             ("pt_q0p0k0", [KB, 2 * QB], DT),
                             ("pt_q0p0k1", [KB, 2 * QB], DT),
                             ("zsb_q0", [64, 6 * QB], DT),
                             ("den_q0", [1, 6 * QB], F32),
                             ("zp_q0", [128, 3 * QB], DT)]:
            d_dbg[nm] = nc.dram_tensor(nm, shp, dtp,
                                       kind="ExternalOutput").ap()

    Exp = mybir.ActivationFunctionType.Exp
    Iden = mybir.ActivationFunctionType.Identity

    with tile.TileContext(nc) as tc:
        with tc.tile_pool(name="persist", bufs=1) as pp, \
             tc.tile_pool(name="work", bufs=4) as wp, \
             tc.tile_pool(name="zsb", bufs=3) as zp, \
             tc.tile_pool(name="outsb", bufs=4) as op, \
             tc.tile_pool(name="psA", bufs=1, space="PSUM") as psA:

            # ---- input DMAs --------------------------------------------------
            # The two first-needed pieces land in parallel: x quarter-0 heads
            # the sync HWDGE ring while wk0/bundle/wq0 head the gpsimd SWDGE
            # queue. The scalar queue issues no DMAs so ACT is never
            # head-of-line blocked behind ring flow control.
            wk0 = pp.tile([128, E], DT, tag="wk0", name="wk0")
            nc.gpsimd.dma_start(wk0[:], d_wk[0:128, :])
            bundle = pp.tile([128, 12], F32, tag="bundle", name="bundle")
            nc.gpsimd.dma_start(bundle[:], d_bundle[:, :])
            wq0 = pp.tile([128, E], DT, tag="wq0", name="wq0")
            nc.gpsimd.dma_start(wq0[:], d_wq[0:128, :])
            wk12 = pp.tile([128, 2 * E], DT, tag="wk12", name="wk12")
            nc.gpsimd.dma_start(
                wk12[:].rearrange("p (c e) -> p c e", c=2),
                d_wk[128:384, :].rearrange("(c p) e -> p c e", p=128))
            wq12 = pp.tile([128, 2 * E], DT, tag="wq12", name="wq12")
            nc.gpsimd.dma_start(
                wq12[:].rearrange("p (c e) -> p c e", c=2),
                d_wq[128:384, :].rearrange("(c p) e -> p c e", p=128))
            masksb = pp.tile([KB, 2 * KB], DT, tag="mask", name="masksb")
            nc.gpsimd.dma_start(masksb[:], d_mask[:, :])
            wv_all = pp.tile([128, VH * HPC * EC], DT, tag="wv", name="wv_all")
            nc.gpsimd.dma_start(wv_all[:], d_wv[:, :])
            wo_all = pp.tile([128, PAIRS * E], DT, tag="wo", name="wo_all")
            nc.gpsimd.dma_start(
                wo_all[:].rearrange("p (c e) -> p c e", c=PAIRS),
                d_wo[:, :].rearrange("(c p) e -> p c e", p=128))

            wk = [wk0, wk12[:, 0:E], wk12[:, E:2 * E]]
            wq = [wq0, wq12[:, 0:E], wq12[:, E:2 * E]]
            wo = [wo_all[:, p * E:(p + 1) * E] for p in range(PAIRS)]
            # partition-0 copy of wo[2] rows 64-127 so pass2's second half can
            # run in PE row group 0 (serialized with the first half — avoids
            # a concurrent-accumulate drain race into the same PSUM)
            wo2hi = pp.tile([128, E], DT, tag="wo2hi", name="wo2hi")
            nc.vector.tensor_copy(wo2hi[0:64, :], wo_all[64:128, 2 * E:3 * E])
            wv = [wv_all[:, e * VH * HPC:(e + 1) * VH * HPC] for e in range(EC)]
            mask3 = masksb[:].rearrange("p (h s) -> p h s", h=2)

            # xT: one DMA per (quarter, half-of-e-chunks); chains depend on
            # 3-chunk halves so the first matmul waits on ~1.2MB, not 3MB.
            xq = [[None, None] for _ in range(4)]
            for quarter in range(4):
                for half in range(2):
                    t = pp.tile([128, 3 * QB], DT, tag=f"xq{quarter}_{half}",
                                name=f"xq{quarter}_{half}")
                    src = d_xT[half * 3 * 128:(half * 3 + 3) * 128,
                               quarter * QB:(quarter + 1) * QB]
                    nc.sync.dma_start(
                        t[:].rearrange("p (c s) -> p c s", c=3),
                        src.rearrange("(c p) s -> p c s", p=128))
                    xq[quarter][half] = t

            # HAM warm-up: ~3.5us of dummy matmuls during the input-DMA wait
            # so the real stream starts at 2.4GHz instead of the cold 1.2.
            warm = pp.tile([128, QB], DT, tag="warm", name="warm")
            nc.vector.memset(warm[:], 0.0)
            for i in range(8):
                wps = psA.tile([128, QB], F32, tag="misc", bufs=2,
                               name=f"warm{i}")
                nc.tensor.matmul(wps[:], warm[:, 0:128], warm[:],
                                 start=True, stop=True)

            def xchunk(e, sb, lo=0, w=QB):
                # [128, w] slice of e-chunk e, query block sb
                base = (e % 3) * QB + lo
                return xq[sb][e // 3][:, base:base + w]

            kt = [[pp.tile([128, QB], DT, tag=f"kt{p}_{sb}", name=f"kt{p}_{sb}")
                   for sb in range(NQB)] for p in range(PAIRS)]
            qt = [[pp.tile([128, QB], DT, tag=f"qt{p}_{sb}", name=f"qt{p}_{sb}")
                   for sb in range(NQB)] for p in range(PAIRS)]
            vt = [pp.tile([128, VW * HPC], DT, tag=f"vt{s}", name=f"vt{s}")
                  for s in range(SC)]
            # ones columns for the denominator trick: memset whole V tiles to
            # 1.0 once; projections only ever write the 64 value columns.
            for s in range(SC):
                nc.vector.memset(vt[s][:], 1.0)

            def _mk_chain():
                def chain(name, width, lhs_of_e, rhs_of_e, copy_out):
                    st = {}
                    def mk(e):
                        def step():
                            if e == 0:
                                st["ps"] = psA.tile(
                                    [128, width], F32, tag="misc", bufs=2,
                                    name=name)
                            nc.tensor.matmul(st["ps"][:],
                                             lhs_of_e(e), rhs_of_e(e),
                                             start=(e == 0), stop=(e == EC - 1))
                        return step
                    for e in range(EC):
                        yield mk(e)
                    yield lambda: copy_out(st["ps"])
                return chain

            def kq_pair_ops(sb, p, chain=None):
                chain = chain or _mk_chain()
                kcopy = lambda ps, p=p, sb=sb: nc.scalar.activation(
                    kt[p][sb][:], ps[:], Iden, bias=bundle[:, 3 + p:4 + p])
                qcopy = lambda ps, p=p, sb=sb: nc.scalar.activation(
                    qt[p][sb][:], ps[:], Iden, bias=bundle[:, p:p + 1])
                yield from chain(
                    f"kps{p}_{sb}", QB,
                    lambda e, p=p: wk[p][:, e * 128:(e + 1) * 128],
                    lambda e, sb=sb: xchunk(e, sb), kcopy)
                yield from chain(
                    f"qps{p}_{sb}", QB,
                    lambda e, p=p: wq[p][:, e * 128:(e + 1) * 128],
                    lambda e, sb=sb: xchunk(e, sb), qcopy)

            def kq_ops(sb, chain=None):
                for p in range(PAIRS):
                    yield from kq_pair_ops(sb, p, chain)

            def v_ops(sb, chain=None):
                chain = chain or _mk_chain()
                for s in range(4 * sb, 4 * sb + 4):
                    def vcopy(ps, s=s):
                        dst = vt[s][:].rearrange(
                            "p (h w) -> p h w", w=VW)[:, :, 0:VH]
                        nc.vector.tensor_copy(
                            dst, ps[:].rearrange("p (h w) -> p h w", w=VH))
                    yield from chain(
                        f"vps{s}", VH * HPC,
                        lambda e, sb=sb, s=s: xchunk(e, sb, (s % 4) * 128, 128),
                        lambda e: wv[e], vcopy)

            def proj_ops(sb):
                yield from kq_ops(sb)
                yield from v_ops(sb)

            def proj_block(sb):
                for step in proj_ops(sb):
                    step()

            def make_normalize(qb, zpair):
                F32R = mybir.dt.float32r
                def normalize(head, zsb, den, last=False):
                    # den is a partition-0 tile: reciprocal_approx_fast is a
                    # custom DVE op that misreads partition-offset inputs on HW
                    p, sub = head // 2, head % 2
                    hsl = slice(sub * 64, sub * 64 + 64)
                    recipf = wp.tile([1, QB], F32, tag="recipf",
                                     name=f"recipf{qb}_{head}")
                    nc.vector.reciprocal_approx_fast(recipf[:], den[:])
                    bcast = wp.tile([64, QB], F32, tag="bcast",
                                    name=f"bcast{qb}_{head}")
                    nc.gpsimd.partition_broadcast(bcast[:], recipf[:])
                    zt = zpair[p]
                    # last pair: per-head tiles (rows 0-63 each) so pass2's
                    # first contraction half starts before head 5 normalizes
                    dst = zt[sub][0:64, :] if isinstance(zt, list) else zt[hsl, :]
                    nc.vector.tensor_mul(dst, zsb[0:64, :], bcast[:])
                return normalize

            def attention(qb, drain=None, late=None, last_pair_drain=None,
                          zpair_override=None):
                nkb = 4 * qb + 4
                dq = list(drain) if drain is not None else []
                iters = [PAIRS * max(nkb - 1, 1), 0]

                def drain_some():
                    if not dq:
                        return
                    n = max(1, -(-len(dq) // max(iters[0] - iters[1], 1)))
                    for _ in range(n):
                        if dq:
                            dq.pop(0)()
                    iters[1] += 1
                zpair = zpair_override or [
                    zp.tile([128, QB], DT, tag=f"zp{p}", name=f"zp{p}_{qb}")
                    for p in range(PAIRS)]
                normalize = make_normalize(qb, zpair)
                pending = []
                for p in range(PAIRS):
                    zab = [psA.tile([VW, QB], F32, tag="z", bufs=2,
                                    name=f"zps{qb}_{2 * p + s}") for s in range(2)]

                    def qk(kb):
                        # both heads of the pair, concurrent via PE row groups;
                        # on diagonal blocks only the causally-live columns.
                        o = kb - 4 * qb
                        lo = o * 128 if o > 0 else 0
                        sps = psA.tile([KB, 2 * QB], F32, tag="s", bufs=2,
                                       name=f"sps{qb}_{p}_{kb}")
                        ktt = kt[p][kb // 4]
                        ksl = slice((kb % 4) * KB, (kb % 4 + 1) * KB)
                        nc.tensor.matmul(
                            sps[:, lo:QB], ktt[0:64, ksl],
                            qt[p][qb][0:64, lo:QB],
                            start=True, stop=True, tile_position=(0, 0))
                        nc.tensor.matmul(
                            sps[:, QB + lo:2 * QB], ktt[64:128, ksl],
                            qt[p][qb][64:128, lo:QB],
                            start=True, stop=True, tile_position=(64, 0))
                        return sps, lo

                    def pv(kb, sps, lo):
                        o = kb - 4 * qb
                        pt = wp.tile([KB, 2 * QB], DT, tag="p", bufs=6,
                                     name=f"pt{qb}_{p}_{kb}")
                        if lo == 0:
                            nc.scalar.activation(pt[:], sps[:], Exp,
                                                 scale=float(SCALE))
                        else:
                            s3 = sps[:].rearrange(
                                "p (h s) -> p h s", h=2)[:, :, lo:]
                            p3 = pt[:].rearrange(
                                "p (h s) -> p h s", h=2)[:, :, lo:]
                            nc.scalar.activation(p3, s3, Exp,
                                                 scale=float(SCALE))
                        if o >= 0:  # diagonal: mask the 128-wide tri chunk
                            pm = pt[:].rearrange(
                                "p (h s) -> p h s", h=2)[:, :, o * 128:(o + 1) * 128]
                            nc.vector.tensor_mul(pm, pm, mask3)
                        if DEBUG and qb == 0 and p == 0 and kb in (0, 1):
                            nc.sync.dma_start(d_dbg[f"pt_q0p0k{kb}"][:, :],
                                              pt[:])
                        for s in range(2):
                            nc.tensor.matmul(
                                zab[s][:, lo:QB],
                                vt[kb][:, (2 * p + s) * VW:(2 * p + s + 1) * VW],
                                pt[:, s * QB + lo:(s + 1) * QB],
                                start=(kb == 0), stop=(kb == nkb - 1))

                    prev = qk(0)
                    for kb in range(1, nkb):
                        cur = qk(kb)
                        pv(kb - 1, *prev)
                        drain_some()
                        prev = cur
                        if kb == 2:
                            for args in pending:
                                normalize(*args)
                            pending = []
                            if p == PAIRS - 1 and last_pair_drain is not None:
                                dq.extend(last_pair_drain)
                    pv(nkb - 1, *prev)
                    drain_some()

                    last = (qb == NQB - 1 and p == PAIRS - 1)
                    zsbs, dens = [], []
                    for s in range(2):
                        head = 2 * p + s
                        zsbs.append(wp.tile([VW, QB], DT, tag="zc", bufs=6,
                                            name=f"zsb{qb}_{head}"))
                        dens.append(wp.tile([1, QB], F32, tag="den", bufs=6,
                                            name=f"den{qb}_{head}"))
                    # one [65,512] bf16 copy per head releases the PSUM zab
                    # ~1.5us sooner (next pair PV is gated on it); the f32 den
                    # row is extracted from SBUF off that critical path
                    if last:
                        # tail: ACT is free once the exp stream ends
                        for s in range(2):
                            nc.scalar.activation(zsbs[s][:], zab[s][:], Iden)
                    else:
                        for s in range(2):
                            nc.vector.tensor_copy(zsbs[s][:], zab[s][:])
                    for s in range(2):
                        nc.vector.tensor_copy(dens[s][:], zsbs[s][64:65, :])
                    for s in range(2):
                        head = 2 * p + s
                        if DEBUG and qb == 0:
                            nc.sync.dma_start(
                                d_dbg["zsb_q0"][:, head * QB:(head + 1) * QB],
                                zsbs[s][:])
                            nc.sync.dma_start(
                                d_dbg["den_q0"][:, head * QB:(head + 1) * QB],
                                dens[s][:])
                        pending.append((head, zsbs[s], dens[s], last))
                for args in pending:
                    normalize(*args)
                if DEBUG and qb == 0:
                    nc.sync.dma_start(d_dbg["kt00"][:, :], kt[0][0][:])
                    nc.sync.dma_start(d_dbg["qt00"][:, :], qt[0][0][:])
                    nc.sync.dma_start(d_dbg["vt0"][:, :], vt[0][:])
                    for p_ in range(PAIRS):
                        nc.sync.dma_start(
                            d_dbg["zp_q0"][:, p_ * QB:(p_ + 1) * QB],
                            zpair[p_][:])
                while dq:
                    dq.pop(0)()
                if late is not None:
                    for step in late:
                        step()
                return outproj_ops(qb, zpair)

            def outproj_split(qb, zpair):
                """qb=3 variant: p0+p1 partials run early (PE bubbles during
                the last pair), only the short p2 pass waits on the final
                normalize."""
                qsl = slice(qb * QB, (qb + 1) * QB)
                partial = [None] * EC

                def pass1():
                    for e in range(EC):
                        st = {}
                        def mk(e, p, st=st):
                            def step():
                                if p == 0:
                                    st["ps"] = psA.tile(
                                        [128, QB], F32, tag="misc", bufs=2,
                                        name=f"opsa{qb}_{e}")
                                nc.tensor.matmul(
                                    st["ps"][:], wo[p][:, e * 128:(e + 1) * 128],
                                    zpair[p][:], start=(p == 0), stop=(p == 1))
                            return step
                        yield mk(e, 0)
                        yield mk(e, 1)
                        def fin(e, st=st):
                            def step():
                                t = op.tile([128, QB], F32, tag=f"partial{e}",
                                            bufs=1, name=f"partial{qb}_{e}")
                                partial[e] = t
                                nc.vector.tensor_copy(t[:], st["ps"][:])
                            return step
                        yield fin(e)

                def pass2():
                    # split by head: the head-4 half of the contraction runs
                    # while head 5 is still normalizing
                    sts = {}
                    def mka(e):
                        def step():
                            sts[e] = psA.tile([128, QB], F32, tag="misc",
                                              bufs=2, name=f"opsb{qb}_{e}")
                            nc.tensor.matmul(
                                sts[e][:], wo[2][0:64, e * 128:(e + 1) * 128],
                                zpair[2][0][0:64, :], start=True, stop=False,
                                tile_position=(0, 0))
                        return step
                    def mkb(e):
                        def step():
                            nc.tensor.matmul(
                                sts[e][:], wo2hi[0:64, e * 128:(e + 1) * 128],
                                zpair[2][1][0:64, :], start=False, stop=True,
                                tile_position=(0, 0))
                        return step
                    def fin(e):
                        def step():
                            osb = op.tile([128, QB], F32, tag="osb",
                                          name=f"osb{qb}_{e}")
                            nc.vector.scalar_tensor_tensor(
                                osb[:], sts[e][:], bundle[:, 6 + e:7 + e],
                                partial[e][:],
                                op0=mybir.AluOpType.add,
                                op1=mybir.AluOpType.add)
                            nc.sync.dma_start(
                                d_out[e * 128:(e + 1) * 128, qsl], osb[:])
                        return step
                    yield mka(0)
                    yield mka(1)
                    for e in range(EC):
                        yield mkb(e)
                        yield fin(e)
                        if e + 2 < EC:
                            yield mka(e + 2)
                return pass1, pass2

            def outproj_ops(qb, zpair):
                qsl = slice(qb * QB, (qb + 1) * QB)
                for e in range(EC):
                    st = {}
                    def mk(e, p):
                        def step():
                            if p == 0:
                                st["ps"] = psA.tile([128, QB], F32, tag="misc",
                                                    bufs=2, name=f"ops{qb}_{e}")
                            nc.tensor.matmul(
                                st["ps"][:], wo[p][:, e * 128:(e + 1) * 128],
                                zpair[p][:],
                                start=(p == 0), stop=(p == PAIRS - 1))
                        return step
                    for p in range(PAIRS):
                        yield mk(e, p)
                    def fin(e):
                        def step():
                            osb = op.tile([128, QB], F32, tag="osb",
                                          name=f"osb{qb}_{e}")
                            nc.vector.tensor_scalar_add(
                                osb[:], st["ps"][:], bundle[:, 6 + e:7 + e])
                            nc.sync.dma_start(d_out[e * 128:(e + 1) * 128, qsl],
                                              osb[:])
                        return step
                    yield fin(e)

            proj_block(0)
            carry = []
            for qb in range(NQB):
                if qb + 1 < NQB:
                    drain = list(carry) + list(kq_ops(qb + 1))
                    oops = attention(qb, drain=iter(drain), late=v_ops(qb + 1))
                    carry = list(oops)
                else:
                    zpair_last = [zp.tile([128, QB], DT, tag=f"zp{p}",
                                          name=f"zpL{p}") for p in range(2)]
                    zpair_last.append(
                        [zp.tile([128, QB], DT, tag="zp2h", bufs=2,
                                 name=f"zpL2_{s}") for s in range(2)])
                    pass1, pass2 = outproj_split(qb, zpair_last)
                    attention(qb, drain=iter(carry),
                              last_pair_drain=pass1(),
                              zpair_override=zpair_last)
                    for step in pass2():
                        step()

    nc.compile()
    return nc


def _get_nc():
    if _g["nc"] is None:
        _g["nc"] = _build()
    return _g["nc"]


def _make_in_maps(inputs):
    x = np.asarray(inputs["normalized_resid_pre"], dtype=np.float32)
    W_Q = np.asarray(inputs["W_Q"], dtype=np.float32)
    W_K = np.asarray(inputs["W_K"], dtype=np.float32)
    W_V = np.asarray(inputs["W_V"], dtype=np.float32)
    W_O = np.asarray(inputs["W_O"], dtype=np.float32)
    b_Q = np.asarray(inputs["b_Q"], dtype=np.float32)
    b_K = np.asarray(inputs["b_K"], dtype=np.float32)
    b_V = np.asarray(inputs["b_V"], dtype=np.float32)
    b_O = np.asarray(inputs["b_O"], dtype=np.float32)
    dt = _np_dt()

    # 0/1 keep-mask for the 128-wide diagonal triangle chunk, duplicated for
    # the two heads of a pair: keep when k-within-chunk <= q-within-chunk.
    tri = np.tril(np.ones((KB, KB), dtype=np.float32)).T  # [dk, dq] keep dk<=dq
    mask = np.concatenate([tri, tri], axis=1).astype(dt)  # [128, 256]

    in_maps = []
    for c in range(8):
        b = c // 2
        hs = (c % 2) * HPC
        heads = list(range(hs, hs + HPC))
        def pack(w):
            # [E, C] -> [128, EC*C] with column block e holding rows e*128..
            C = w.shape[1]
            return np.ascontiguousarray(
                w.reshape(EC, 128, C).transpose(1, 0, 2).reshape(128, EC * C))

        wq = np.concatenate(
            [pack(np.concatenate([W_Q[heads[2 * p]], W_Q[heads[2 * p + 1]]], axis=1))
             for p in range(PAIRS)], axis=0)             # [3*128, 768]
        wk = np.concatenate(
            [pack(np.concatenate([W_K[heads[2 * p]], W_K[heads[2 * p + 1]]], axis=1))
             for p in range(PAIRS)], axis=0)
        wv = pack(np.concatenate([W_V[h] for h in heads], axis=1))  # [128, 6*384]
        wo = np.concatenate(
            [np.concatenate([W_O[heads[2 * p]], W_O[heads[2 * p + 1]]], axis=0)
             for p in range(PAIRS)], axis=0)             # [3*128, 768]

        # bias bundle [128, 12]: cols 0-2 bQ pairs, 3-5 bK pairs, 6-11 bO_eff
        bundle = np.zeros((128, 12), dtype=np.float32)
        for p in range(PAIRS):
            bundle[:, p] = np.concatenate(
                [b_Q[heads[2 * p]], b_Q[heads[2 * p + 1]]])
            bundle[:, 3 + p] = np.concatenate(
                [b_K[heads[2 * p]], b_K[heads[2 * p + 1]]])
        # fold b_V into b_O: out += sum_h Wo[h] @ bV[h]  (sum(P)/den == 1)
        bo_eff = b_O / 2.0 + np.einsum(
            "nhe,nh->e", W_O[heads], b_V[heads]).astype(np.float32)
        bundle[:, 6:12] = bo_eff.reshape(EC, 128).T

        in_maps.append({
            "xT": np.ascontiguousarray(x[b].T).astype(dt),
            "wq": wq.astype(dt), "wk": wk.astype(dt),
            "wv": wv.astype(dt), "wo": wo.astype(dt),
            "bundle": bundle,
            "mask": mask,
        })
    return in_maps


def _gather(results):
    out = np.empty((B, S, E), dtype=np.float32)
    for b in range(B):
        acc = results[2 * b]["outT"].astype(np.float32) + \
              results[2 * b + 1]["outT"].astype(np.float32)
        out[b] = acc.T
    return out


def run(inputs, trace=False):
    """Returns (output, BassKernelResults)."""
    from concourse.bass_utils import run_bass_kernel_spmd

    if trace:
        _install_ntff_shim()
    nc = _get_nc()
    in_maps = _make_in_maps(inputs)
    res = run_bass_kernel_spmd(nc, in_maps, core_ids=list(range(8)), trace=trace)
    return _gather(res.results), res


def kernel(**inputs):
    out, _ = run(inputs, trace=False)
    return out


def _install_ntff_shim():
    """The agent image's antenv lacks axon_hooks; recreate it so
    run_bass_kernel_spmd(trace=True) can capture NTFF profiles."""
    import types, ctypes, contextlib

    if "antenv.axon_hooks" in sys.modules:
        return
    so_path = "/opt/axon/libaxon_pjrt.so"
    try:
        lib = ctypes.CDLL(so_path)
        lib.axon_start_nrt_profile.argtypes = [ctypes.POINTER(ctypes.c_int64),
                                              ctypes.c_size_t]
        lib.axon_start_nrt_profile.restype = ctypes.c_int64
        lib.axon_stop_nrt_profile.argtypes = [ctypes.c_char_p]
        lib.axon_stop_nrt_profile.restype = ctypes.c_int64
    except (OSError, AttributeError):
        return

    @contextlib.contextmanager
    def _hook(output_dir, device_ids):
        import jax

        jax.devices()
        if device_ids:
            ids = (ctypes.c_int64 * len(device_ids))(*device_ids)
            rc = lib.axon_start_nrt_profile(ids, len(device_ids))
        else:
            rc = lib.axon_start_nrt_profile(None, 0)
        if rc != 0:
            raise RuntimeError(f"axon_start_nrt_profile rc={rc}")
        try:
            yield
        finally:
            n = lib.axon_stop_nrt_profile(str(output_dir).encode())
            print(f"ntff profile: {n} file(s) -> {output_dir}", file=sys.stderr)

    mod = types.ModuleType("antenv.axon_hooks")
    mod.get_axon_ntff_profile_hook = lambda: _hook
    sys.modules["antenv.axon_hooks"] = mod
    # avoid S3 upload attempts from the trace post-processing
    from concourse import bass_utils as bu

    bu.upload_artifacts = lambda tmpdir: f"local:{tmpdir}"
